# revision 1
# baseline (speedup 1.0000x reference)
"""Distributed Bass kernel for nn_Interaction_GraphConvolution.

Math (reference):
    x  = node_features @ linear_w.T + linear_b          [N, IN_F]
    wf = x @ weight                                     [N, C]
    G  = mask_father[:,0,:].T @ adjacency               [N, N]
    P  = G * mask_hadamard[:,0,:].T                     [N, N]
    out[c, j] = wf[j,c] * (P @ wf)[j,c] / neighbor_count[c]^2

Sharding: output columns j (node dim) split across 8 cores, 512 each.
Two SPMD launches:
  NEFF-1: core m computes wf rows J_m (512 rows). Host gathers full wf.
  NEFF-2: core m computes G^T/P^T columns J_m and out[:, J_m].
Dtypes: adjacency-side matmuls in bf16 (inputs are small ints - exact);
wf-side matmuls in float32r (~1.5e-4 rel err at full PE rate).
"""

import os
import sys

sys.path.insert(0, "/opt/trn_rl_repo")

import numpy as np
import ml_dtypes

from concourse import bass, bacc, mybir, tile
from concourse.bass_utils import run_bass_kernel_spmd
from concourse.masks import make_identity

F32 = mybir.dt.float32
F32R = mybir.dt.float32r
BF16 = mybir.dt.bfloat16

N = 4096       # nodes (== out channels C)
F_RAW = 512    # raw feature dim
IN_F = 1024    # hidden dim
C = 4096       # out channels
M = 8          # cores
JB = N // M    # 512 output columns per core

LAST_EXEC = {}
LAST_RESULTS = {}


def _build_neff1():
    """Per core: wf_rows[J_m] = (nf[J_m] @ lw.T + b) @ W, via transposed tiles.

    Inputs (per core): lwT [F_RAW, IN_F] f32r, nfT [F_RAW, JB] f32r,
    bias [128, IN_F//128] f32, w [IN_F, C] f32r.
    Output: wf_rows [JB, C] f32.
    """
    nc = bacc.Bacc()
    lwT_d = nc.dram_tensor("lwT", [F_RAW, IN_F], F32R, kind="ExternalInput")
    nfT_d = nc.dram_tensor("nfT", [F_RAW, JB], F32R, kind="ExternalInput")
    b_d = nc.dram_tensor("bias", [128, IN_F // 128], F32, kind="ExternalInput")
    w_d = nc.dram_tensor("w", [IN_F, C], F32R, kind="ExternalInput")
    wf_d = nc.dram_tensor("wf_rows", [JB, C], F32, kind="ExternalOutput")

    NFB = IN_F // 128   # 8 f-blocks
    NRB = F_RAW // 128  # 4 r-blocks
    NJB = JB // 128     # 4 j-blocks
    NCC = C // 512      # 8 c-chunks

    with tile.TileContext(nc) as tc:
        with tc.tile_pool(name="const", bufs=1) as constp:
            lwT_t = constp.tile([128, NRB * IN_F], F32R)
            for rb in range(NRB):
                nc.sync.dma_start(
                    lwT_t[:, rb * IN_F:(rb + 1) * IN_F],
                    lwT_d[rb * 128:(rb + 1) * 128, :])
            nfT_t = constp.tile([128, NRB * JB], F32R)
            for rb in range(NRB):
                nc.sync.dma_start(
                    nfT_t[:, rb * JB:(rb + 1) * JB],
                    nfT_d[rb * 128:(rb + 1) * 128, :])
            b_t = constp.tile([128, NFB], F32)
            nc.sync.dma_start(b_t[:], b_d[:])
            w_t = constp.tile([128, NFB * C], F32R)
            for fb in range(NFB):
                nc.sync.dma_start(
                    w_t[:, fb * C:(fb + 1) * C],
                    w_d[fb * 128:(fb + 1) * 128, :])
            xt_t = constp.tile([128, NFB * JB], F32R)

            # phase X: xT[f, j] = lw @ nf[J_m].T + b
            with tc.tile_pool(name="psx", bufs=2, space=bass.MemorySpace.PSUM) as psxp:
                for fb in range(NFB):
                    psx = psxp.tile([128, JB], F32, tag="psx")
                    for rb in range(NRB):
                        nc.tensor.matmul(
                            psx[:],
                            lwT_t[:, rb * IN_F + fb * 128: rb * IN_F + (fb + 1) * 128],
                            nfT_t[:, rb * JB:(rb + 1) * JB],
                            start=(rb == 0), stop=(rb == NRB - 1))
                    nc.scalar.activation(
                        xt_t[:, fb * JB:(fb + 1) * JB], psx[:],
                        mybir.ActivationFunctionType.Identity,
                        bias=b_t[:, fb:fb + 1], scale=1.0)

            # phase W: wf[J_m] = xT.T @ W
            with tc.tile_pool(name="psw", bufs=8, space=bass.MemorySpace.PSUM) as pswp, \
                 tc.tile_pool(name="io1", bufs=3) as iop:
                for jb in range(NJB):
                    for cc in range(NCC):
                        pw = pswp.tile([128, 512], F32, tag="pw")
                        for fb in range(NFB):
                            nc.tensor.matmul(
                                pw[:],
                                xt_t[:, fb * JB + jb * 128: fb * JB + (jb + 1) * 128],
                                w_t[:, fb * C + cc * 512: fb * C + (cc + 1) * 512],
                                start=(fb == 0), stop=(fb == NFB - 1))
                        o_sb = iop.tile([128, 512], F32, tag="o_sb")
                        nc.vector.tensor_copy(o_sb[:], pw[:])
                        nc.sync.dma_start(
                            wf_d[jb * 128:(jb + 1) * 128, cc * 512:(cc + 1) * 512],
                            o_sb[:])
    nc.finalize()
    return nc


def _build_neff2():
    """Per core: G^T/P^T for columns J_m, then out[:, J_m].

    Inputs: a [N, N] bf16 (adjacency), ao [N, JB] bf16 (mask_father cols),
    s [N, JB] bf16 (mask_hadamard cols), wfd [N, C] f32r (full wf),
    wfs [JB, C] f32 (wf rows J_m, pre-scaled by nothing - raw),
    inv2 [128, N//128] f32 (1/neighbor_count^2 tiled).
    Output: outc [C, JB] f32  (= output[:, J_m]).
    """
    nc = bacc.Bacc()
    a_d = nc.dram_tensor("a", [N, N], BF16, kind="ExternalInput")
    ao_d = nc.dram_tensor("ao", [N, JB], BF16, kind="ExternalInput")
    s_d = nc.dram_tensor("s", [N, JB], BF16, kind="ExternalInput")
    wf_d = nc.dram_tensor("wfd", [N, C], F32R, kind="ExternalInput")
    wr_d = nc.dram_tensor("wfs", [JB, C], F32, kind="ExternalInput")
    i2_d = nc.dram_tensor("inv2", [128, N // 128], F32, kind="ExternalInput")
    out_d = nc.dram_tensor("outc", [C, JB], F32, kind="ExternalOutput")

    NKB = N // 128    # 32 k-blocks
    NIB = N // 128    # 32 i-blocks
    NCB = C // 128    # 32 c-blocks
    NJB = JB // 128   # 4 j-blocks

    with tile.TileContext(nc) as tc:
        with tc.tile_pool(name="const", bufs=1) as constp:
            ident = constp.tile([128, 128], F32)
            make_identity(nc, ident[:])
            i2_t = constp.tile([128, N // 128], F32)
            nc.sync.dma_start(i2_t[:], i2_d[:])
            aot = constp.tile([128, NKB * JB], BF16)
            for kb in range(NKB):
                nc.sync.dma_start(
                    aot[:, kb * JB:(kb + 1) * JB],
                    ao_d[kb * 128:(kb + 1) * 128, :])
            pt_t = constp.tile([128, NIB * JB], F32R)

            # phase G: PT[i, j] = (A^T @ Ao) * S  for j in J_m
            with tc.tile_pool(name="psg", bufs=8, space=bass.MemorySpace.PSUM) as psgp, \
                 tc.tile_pool(name="ioa", bufs=3) as ioa, \
                 tc.tile_pool(name="ios", bufs=2) as ios:
                for isup in range(NIB // 8):
                    psg = [psgp.tile([128, JB], F32, tag="psg", name=f"psg{_i}") for _i in range(8)]
                    for kb in range(NKB):
                        a_t = ioa.tile([128, 1024], BF16, tag="a_t")
                        nc.sync.dma_start(
                            a_t[:],
                            a_d[kb * 128:(kb + 1) * 128,
                                isup * 1024:(isup + 1) * 1024])
                        for ib8 in range(8):
                            nc.tensor.matmul(
                                psg[ib8][:],
                                a_t[:, ib8 * 128:(ib8 + 1) * 128],
                                aot[:, kb * JB:(kb + 1) * JB],
                                start=(kb == 0), stop=(kb == NKB - 1))
                    for ib8 in range(8):
                        ib = isup * 8 + ib8
                        s_t = ios.tile([128, JB], BF16, tag="s_t")
                        nc.sync.dma_start(s_t[:], s_d[ib * 128:(ib + 1) * 128, :])
                        nc.vector.tensor_mul(
                            pt_t[:, ib * JB:(ib + 1) * JB], psg[ib8][:], s_t[:])

            # phase O: out[c, j] = (wf^T @ PT) * wf^T * inv2
            with tc.tile_pool(name="pso", bufs=4, space=bass.MemorySpace.PSUM) as psop, \
                 tc.tile_pool(name="pst", bufs=2, space=bass.MemorySpace.PSUM) as pstp, \
                 tc.tile_pool(name="iow", bufs=4) as iow, \
                 tc.tile_pool(name="ior", bufs=4) as ior, \
                 tc.tile_pool(name="ioo", bufs=3) as ioo:
                for csup in range(NCB // 4):
                    pso = [psop.tile([128, JB], F32, tag="pso", name=f"pso{_i}") for _i in range(4)]
                    for ib in range(NIB):
                        wf_t = iow.tile([128, 512], F32R, tag="wf_t")
                        nc.sync.dma_start(
                            wf_t[:],
                            wf_d[ib * 128:(ib + 1) * 128,
                                 csup * 512:(csup + 1) * 512])
                        for cb4 in range(4):
                            nc.tensor.matmul(
                                pso[cb4][:],
                                wf_t[:, cb4 * 128:(cb4 + 1) * 128],
                                pt_t[:, ib * JB:(ib + 1) * JB],
                                start=(ib == 0), stop=(ib == NIB - 1))
                    for cb4 in range(4):
                        cb = csup * 4 + cb4
                        ptp = pstp.tile([128, JB], F32, tag="ptp")
                        for jb in range(NJB):
                            wr_t = ior.tile([128, 128], F32, tag="wr_t")
                            nc.sync.dma_start(
                                wr_t[:],
                                wr_d[jb * 128:(jb + 1) * 128,
                                     cb * 128:(cb + 1) * 128])
                            nc.tensor.transpose(
                                ptp[:, jb * 128:(jb + 1) * 128], wr_t[:], ident[:])
                        wt_sb = ioo.tile([128, JB], F32, tag="wt_sb")
                        nc.scalar.activation(
                            wt_sb[:], ptp[:],
                            mybir.ActivationFunctionType.Identity,
                            bias=0.0, scale=i2_t[:, cb:cb + 1])
                        o_sb = ioo.tile([128, JB], F32, tag="o_sb")
                        nc.vector.tensor_mul(o_sb[:], pso[cb4][:], wt_sb[:])
                        nc.sync.dma_start(out_d[cb * 128:(cb + 1) * 128, :], o_sb[:])
    nc.finalize()
    return nc


_NC1 = None
_NC2 = None


def _get_ncs():
    global _NC1, _NC2
    if _NC1 is None:
        _NC1 = _build_neff1()
        _NC2 = _build_neff2()
    return _NC1, _NC2


def _ensure_trace_hook():
    """Best-effort NTFF profiling shim (test harness only; grading runs
    without tracing). The agent image's antenv lacks axon_hooks, but the
    axon boot package exposes the ctypes equivalent."""
    try:
        from antenv.axon_hooks import get_axon_ntff_profile_hook
        return get_axon_ntff_profile_hook() is not None
    except ImportError:
        pass
    try:
        import types
        if "/root/.axon_site" not in sys.path:
            sys.path.insert(0, "/root/.axon_site")
        from trn_agent_boot.trn_boot import _ntff_profile_via_ctypes
        hook = _ntff_profile_via_ctypes("/opt/axon/libaxon_pjrt.so")
        if hook is None:
            return False
        import antenv
        mod = types.ModuleType("antenv.axon_hooks")
        mod.get_axon_ntff_profile_hook = lambda: hook
        mod.set_axon_ntff_profile_hook = lambda h: None
        sys.modules["antenv.axon_hooks"] = mod
        antenv.axon_hooks = mod
        from concourse import bass_utils as _bu
        _bu.upload_artifacts = lambda tmpdir: ""
        return True
    except Exception:
        return False


def _run(nc, in_maps, cores, trace, tag):
    if trace:
        try:
            r = run_bass_kernel_spmd(nc, in_maps, cores, trace=True)
            LAST_EXEC[tag] = r.exec_time_ns
            LAST_RESULTS[tag] = r
            return r
        except Exception as e:
            print(f"trace run failed ({e!r}); retrying without trace")
    return run_bass_kernel_spmd(nc, in_maps, cores)


def kernel(node_features, adjacency_matrix, mask_father, neighbor_count,
           mask_hadamard, linear_w, linear_b, weight):
    nc1, nc2 = _get_ncs()
    trace = bool(int(os.environ.get("BASS_KERNEL_TRACE", "0"))) and _ensure_trace_hook()
    cores = list(range(M))
    bf = ml_dtypes.bfloat16

    nf = np.ascontiguousarray(np.asarray(node_features, dtype=np.float32))
    A = np.ascontiguousarray(np.asarray(adjacency_matrix, dtype=np.float32))
    Ao = np.ascontiguousarray(np.asarray(mask_father, dtype=np.float32)[:, 0, :])
    S = np.ascontiguousarray(np.asarray(mask_hadamard, dtype=np.float32)[:, 0, :])
    ncnt = np.asarray(neighbor_count, dtype=np.float32)
    lw = np.asarray(linear_w, dtype=np.float32)
    lb = np.asarray(linear_b, dtype=np.float32)
    W = np.ascontiguousarray(np.asarray(weight, dtype=np.float32))

    # ---- launch 1: wf rows ----
    lwT = np.ascontiguousarray(lw.T)                       # [F_RAW, IN_F]
    bias = np.ascontiguousarray(lb.reshape(IN_F // 128, 128).T)  # [128, 8]
    in1 = []
    for m in range(M):
        nfT = np.ascontiguousarray(nf[m * JB:(m + 1) * JB, :].T)  # [F_RAW, JB]
        in1.append({"lwT": lwT, "nfT": nfT, "bias": bias, "w": W})
    r1 = _run(nc1, in1, cores, trace, "neff1")
    wf = np.concatenate([r1.results[m]["wf_rows"] for m in range(M)], axis=0)

    # ---- launch 2: graph conv ----
    A_b = A.astype(bf)
    inv2 = (1.0 / np.square(ncnt.astype(np.float64)))[:, 0].astype(np.float32)
    inv2_t = np.ascontiguousarray(inv2.reshape(N // 128, 128).T)  # [128, 32]
    in2 = []
    for m in range(M):
        sl = slice(m * JB, (m + 1) * JB)
        in2.append({
            "a": A_b,
            "ao": np.ascontiguousarray(Ao[:, sl]).astype(bf),
            "s": np.ascontiguousarray(S[:, sl]).astype(bf),
            "wfd": wf,
            "wfs": np.ascontiguousarray(wf[sl, :]),
            "inv2": inv2_t,
        })
    r2 = _run(nc2, in2, cores, trace, "neff2")

    out = np.empty((C, N), dtype=np.float32)
    for m in range(M):
        out[:, m * JB:(m + 1) * JB] = r2.results[m]["outc"]
    return out



# revision 10
# speedup vs baseline: 1.9350x; 1.9350x over previous
"""Distributed Bass kernel for nn_Interaction_GraphConvolution.

Math (reference):
    x  = node_features @ linear_w.T + linear_b          [N, IN_F]
    wf = x @ weight                                     [N, C]
    G  = mask_father[:,0,:].T @ adjacency               [N, N]
    P  = G * mask_hadamard[:,0,:].T                     [N, N]
    out[c, j] = wf[j,c] * (P @ wf)[j,c] / neighbor_count[c]^2

Sharding: output columns j (node dim) split across 8 cores, 512 each.
Two SPMD launches:
  NEFF-1: core m computes wf rows J_m (512 rows). Host gathers full wf.
  NEFF-2: core m computes G^T/P^T columns J_m and out[:, J_m].

Dtypes: adjacency-side matmul in fp8-e4m3 DoubleRow (inputs are 0/1 ints -
exact, 2x PE rate); wf-side matmuls in bf16; the final elementwise wf^T
factor stays f32 with 1/neighbor_count^2 folded in on the host.
All DRAM operands are host-packed so each DMA moves a multi-KB contiguous
line per partition (few large DMAs instead of hundreds of small ones).
"""

import os
import sys

sys.path.insert(0, "/opt/trn_rl_repo")

import numpy as np
import ml_dtypes

from concourse import bass, bacc, mybir, tile
from concourse.bass_utils import run_bass_kernel_spmd

F32 = mybir.dt.float32
F32R = mybir.dt.float32r
BF16 = mybir.dt.bfloat16
FP8 = mybir.dt.float8e4
DR = mybir.MatmulPerfMode.DoubleRow

BF = ml_dtypes.bfloat16
F8 = ml_dtypes.float8_e4m3fn

N = 4096       # nodes (== out channels C)
F_RAW = 512    # raw feature dim
IN_F = 1024    # hidden dim
C = 4096       # out channels
M = 8          # cores
JB = N // M    # 512 output columns per core

LAST_EXEC = {}
LAST_RESULTS = {}


def _build_neff1():
    """Per core: wf_rows[J_m] = (nf[J_m] @ lw.T + b) @ W.

    lwT  [128, 4*1024] f32r : lw.T packed (p, rb, f), r = rb*128+p
    nfT  [128, 4*512]  f32r : nf[J_m].T packed (p, rb, j)
    bias [128, 8]      f32  : b packed (p, fb), f = fb*128+p
    w    [128, 8*4096] bf16 : W packed (p, fb, c)
    out wf_rows [JB, C] f32
    """
    nc = bacc.Bacc()
    lwT_d = nc.dram_tensor("lwT", [128, 4 * IN_F], F32R, kind="ExternalInput")
    nfT_d = nc.dram_tensor("nfT", [128, 4 * JB], F32R, kind="ExternalInput")
    b_d = nc.dram_tensor("bias", [128, 8], F32, kind="ExternalInput")
    w_d = nc.dram_tensor("w", [128, 8 * C], BF16, kind="ExternalInput")
    wf_d = nc.dram_tensor("wf_rows", [JB, C], F32, kind="ExternalOutput")

    NRB = 4   # 128-blocks of F_RAW
    NFB = 8   # 128-blocks of IN_F
    NJB = 4   # 128-blocks of JB

    with tile.TileContext(nc) as tc:
        with tc.tile_pool(name="const", bufs=1) as constp, \
             tc.tile_pool(name="psx", bufs=2, space=bass.MemorySpace.PSUM) as psxp, \
             tc.tile_pool(name="psw", bufs=6, space=bass.MemorySpace.PSUM) as pswp, \
             tc.tile_pool(name="io1", bufs=3) as iop:
            lwT_t = constp.tile([128, NRB, IN_F], F32R)
            nc.sync.dma_start(
                lwT_t[:], lwT_d[:].rearrange("p (r f) -> p r f", r=NRB))
            nfT_t = constp.tile([128, NRB, JB], F32R)
            nc.sync.dma_start(
                nfT_t[:], nfT_d[:].rearrange("p (r j) -> p r j", r=NRB))
            b_t = constp.tile([128, NFB], F32)
            nc.sync.dma_start(b_t[:], b_d[:])
            w_t = constp.tile([128, NFB, C], BF16)
            for fb in range(NFB):
                nc.sync.dma_start(w_t[:, fb, :], w_d[:, fb * C:(fb + 1) * C])
            xt_t = constp.tile([128, NFB, JB], BF16)

            # phase X: xT[f, j] = lw @ nf[J_m].T + b  (bf16 out)
            for fb in range(NFB):
                psx = psxp.tile([128, JB], F32, tag="psx")
                for rb in range(NRB):
                    nc.tensor.matmul(
                        psx[:],
                        lwT_t[:, rb, fb * 128:(fb + 1) * 128],
                        nfT_t[:, rb, :],
                        start=(rb == 0), stop=(rb == NRB - 1))
                nc.scalar.activation(
                    xt_t[:, fb, :], psx[:],
                    mybir.ActivationFunctionType.Identity,
                    bias=b_t[:, fb:fb + 1], scale=1.0)

            # phase W: wf[J_m] = xT.T @ W  (bf16 x bf16, 4 psum banks/chunk)
            for jb in range(NJB):
                for ch in range(2):
                    pw = [pswp.tile([128, 512], F32, tag="pw", name=f"pw{i}")
                          for i in range(4)]
                    for fb in range(NFB):
                        for c4 in range(4):
                            nc.tensor.matmul(
                                pw[c4][:],
                                xt_t[:, fb, jb * 128:(jb + 1) * 128],
                                w_t[:, fb, (ch * 4 + c4) * 512:(ch * 4 + c4 + 1) * 512],
                                start=(fb == 0), stop=(fb == NFB - 1))
                    o_sb = iop.tile([128, 2048], F32, tag="o_sb")
                    for c4 in range(4):
                        nc.vector.tensor_copy(
                            o_sb[:, c4 * 512:(c4 + 1) * 512], pw[c4][:])
                    nc.sync.dma_start(
                        wf_d[jb * 128:(jb + 1) * 128,
                             ch * 2048:(ch + 1) * 2048], o_sb[:])
    nc.finalize()
    return nc


def _build_neff2():
    """Per core: PT cols J_m via fp8 DoubleRow, then out[:, J_m] in bf16.

    ap  [128, 8*16*2*512] fp8 : A packed (p, isup, kbb, h, i), k=kbb*256+h*128+p
    aot [128, 16*2*512]   fp8 : Ao[:, J_m] packed (p, kbb, h, j)
    sp  [128, 8*4*512]   bf16 : S[:, J_m] packed (p, isup, ib, j), i=isup*512+ib*128+p
    wfp [128, 8*32*512]  bf16 : wf packed (p, csup, ib, c), i=ib*128+p
    wtp [128, 8*4*512]    f32 : wf.T * inv_ncnt2 packed (p, csup, cb, j), c=csup*512+cb*128+p
    out outc [C, JB] f32
    """
    nc = bacc.Bacc()
    ap_d = nc.dram_tensor("ap", [128, 8 * 16 * 2 * 512], FP8, kind="ExternalInput")
    aot_d = nc.dram_tensor("aot", [128, 16 * 2 * 512], FP8, kind="ExternalInput")
    sp_d = nc.dram_tensor("sp", [128, 8 * 4 * 512], BF16, kind="ExternalInput")
    wfp_d = nc.dram_tensor("wfp", [128, 8 * 32 * 512], BF16, kind="ExternalInput")
    wtp_d = nc.dram_tensor("wtp", [128, 8 * 4 * 512], F32, kind="ExternalInput")
    out_d = nc.dram_tensor("outc", [C, JB], F32, kind="ExternalOutput")

    NIS = 8    # i-supers of 512
    NKBB = 16  # 256-blocks of k
    NCS = 8    # c-supers of 512

    with tile.TileContext(nc) as tc:
        with tc.tile_pool(name="const", bufs=1) as constp, \
             tc.tile_pool(name="ga", bufs=2) as gap, \
             tc.tile_pool(name="gs", bufs=2) as gsp, \
             tc.tile_pool(name="wfpool", bufs=2) as wfpool, \
             tc.tile_pool(name="wtpool", bufs=2) as wtpool, \
             tc.tile_pool(name="oo", bufs=2) as oop:
            aot_t = constp.tile([128, NKBB, 2, 512], FP8)
            nc.sync.dma_start(
                aot_t[:], aot_d[:].rearrange("p (k h j) -> p k h j", k=NKBB, h=2))
            pt_t = constp.tile([128, 32, 512], BF16)

            # phase G: PT[i, j] = (A^T @ Ao) * S  (fp8 DoubleRow, K=256/matmul)
            with tc.tile_pool(name="psg", bufs=8, space=bass.MemorySpace.PSUM) as psgp:
              for isup in range(NIS):
                a_t = gap.tile([128, NKBB, 2, 512], FP8, tag="a_t")
                nc.sync.dma_start(
                    a_t[:],
                    ap_d[:, isup * 16384:(isup + 1) * 16384]
                    .rearrange("p (k h i) -> p k h i", k=NKBB, h=2))
                s_t = gsp.tile([128, 4, 512], BF16, tag="s_t")
                nc.sync.dma_start(
                    s_t[:],
                    sp_d[:, isup * 2048:(isup + 1) * 2048]
                    .rearrange("p (b j) -> p b j", b=4))
                psg = [psgp.tile([128, 512], F32, tag="psg", name=f"psg{i}")
                       for i in range(4)]
                for kbb in range(NKBB):
                    for ib4 in range(4):
                        nc.tensor.matmul(
                            psg[ib4][:],
                            a_t[:, kbb, :, ib4 * 128:(ib4 + 1) * 128],
                            aot_t[:, kbb, :, :],
                            start=(kbb == 0), stop=(kbb == NKBB - 1),
                            perf_mode=DR)
                for ib4 in range(4):
                    nc.vector.tensor_mul(
                        pt_t[:, isup * 4 + ib4, :], psg[ib4][:], s_t[:, ib4, :])

            # phase O: out[c, j] = (wf^T @ PT) * (wf^T * inv2)
            with tc.tile_pool(name="pso", bufs=8, space=bass.MemorySpace.PSUM) as psop:
              for csup in range(NCS):
                wf_t = wfpool.tile([128, 32, 512], BF16, tag="wf_t")
                nc.sync.dma_start(
                    wf_t[:],
                    wfp_d[:, csup * 16384:(csup + 1) * 16384]
                    .rearrange("p (b c) -> p b c", b=32))
                wt_t = wtpool.tile([128, 4, 512], F32, tag="wt_t")
                nc.sync.dma_start(
                    wt_t[:],
                    wtp_d[:, csup * 2048:(csup + 1) * 2048]
                    .rearrange("p (b j) -> p b j", b=4))
                pso = [psop.tile([128, 512], F32, tag="pso", name=f"pso{i}")
                       for i in range(4)]
                for ib in range(32):
                    for cb in range(4):
                        nc.tensor.matmul(
                            pso[cb][:],
                            wf_t[:, ib, cb * 128:(cb + 1) * 128],
                            pt_t[:, ib, :],
                            start=(ib == 0), stop=(ib == 31))
                o_sb = oop.tile([128, 4, 512], F32, tag="o_sb")
                for cb in range(4):
                    nc.vector.tensor_mul(
                        o_sb[:, cb, :], pso[cb][:], wt_t[:, cb, :])
                nc.sync.dma_start(
                    out_d[csup * 512:(csup + 1) * 512, :]
                    .rearrange("(b p) j -> p b j", p=128), o_sb[:])
    nc.finalize()
    return nc


# ---- host-side packing helpers ----

def _pack_neff1_inputs(nf, lw, lb, W):
    lwT = np.ascontiguousarray(
        lw.T.reshape(4, 128, IN_F).transpose(1, 0, 2).reshape(128, -1))
    bias = np.ascontiguousarray(lb.reshape(8, 128).T)
    wp = np.ascontiguousarray(
        W.reshape(8, 128, C).transpose(1, 0, 2).reshape(128, -1).astype(BF))
    in1 = []
    for m in range(M):
        nfT = nf[m * JB:(m + 1) * JB, :].T  # [F_RAW, JB]
        nfp = np.ascontiguousarray(
            nfT.reshape(4, 128, JB).transpose(1, 0, 2).reshape(128, -1))
        in1.append({"lwT": lwT, "nfT": nfp, "bias": bias, "w": wp})
    return in1


def _pack_a_fp8(A):
    # (p, isup, kbb, h, i) with k = kbb*256 + h*128 + p, i = isup*512 + i
    a8 = A.astype(F8)
    return np.ascontiguousarray(
        a8.reshape(16, 2, 128, 8, 512).transpose(2, 3, 0, 1, 4).reshape(128, -1))


def _pack_cols_kh(X, dtype):
    # X [N, JB] -> (p, kbb, h, j) with k = kbb*256 + h*128 + p
    return np.ascontiguousarray(
        X.astype(dtype).reshape(16, 2, 128, JB).transpose(2, 0, 1, 3).reshape(128, -1))


def _pack_rows_sup(X, dtype, nsup, nb):
    # X [N, JB] -> (p, sup, b, j) with row = sup*512 + b*128 + p
    return np.ascontiguousarray(
        X.astype(dtype).reshape(nsup, nb, 128, -1).transpose(2, 0, 1, 3).reshape(128, -1))


_NC1 = None
_NC2 = None


def _get_ncs():
    global _NC1, _NC2
    if _NC1 is None:
        _NC1 = _build_neff1()
        _NC2 = _build_neff2()
    return _NC1, _NC2


def _ensure_trace_hook():
    """Best-effort NTFF profiling shim (test harness only; grading runs
    without tracing). The agent image's antenv lacks axon_hooks, but the
    axon boot package exposes the ctypes equivalent."""
    try:
        from antenv.axon_hooks import get_axon_ntff_profile_hook
        return get_axon_ntff_profile_hook() is not None
    except ImportError:
        pass
    try:
        import types
        if "/root/.axon_site" not in sys.path:
            sys.path.insert(0, "/root/.axon_site")
        from trn_agent_boot.trn_boot import _ntff_profile_via_ctypes
        hook = _ntff_profile_via_ctypes("/opt/axon/libaxon_pjrt.so")
        if hook is None:
            return False
        import antenv
        mod = types.ModuleType("antenv.axon_hooks")
        mod.get_axon_ntff_profile_hook = lambda: hook
        mod.set_axon_ntff_profile_hook = lambda h: None
        sys.modules["antenv.axon_hooks"] = mod
        antenv.axon_hooks = mod
        from concourse import bass_utils as _bu
        _bu.upload_artifacts = lambda tmpdir: ""
        return True
    except Exception:
        return False


def _run(nc, in_maps, cores, trace, tag):
    if trace:
        try:
            r = run_bass_kernel_spmd(nc, in_maps, cores, trace=True)
            LAST_EXEC[tag] = r.exec_time_ns
            LAST_RESULTS[tag] = r
            return r
        except Exception as e:
            print(f"trace run failed ({e!r}); retrying without trace")
    return run_bass_kernel_spmd(nc, in_maps, cores)


def kernel(node_features, adjacency_matrix, mask_father, neighbor_count,
           mask_hadamard, linear_w, linear_b, weight):
    nc1, nc2 = _get_ncs()
    trace = bool(int(os.environ.get("BASS_KERNEL_TRACE", "0"))) and _ensure_trace_hook()
    cores = list(range(M))

    nf = np.ascontiguousarray(np.asarray(node_features, dtype=np.float32))
    A = np.ascontiguousarray(np.asarray(adjacency_matrix, dtype=np.float32))
    Ao = np.ascontiguousarray(np.asarray(mask_father, dtype=np.float32)[:, 0, :])
    S = np.ascontiguousarray(np.asarray(mask_hadamard, dtype=np.float32)[:, 0, :])
    ncnt = np.asarray(neighbor_count, dtype=np.float32)
    lw = np.asarray(linear_w, dtype=np.float32)
    lb = np.asarray(linear_b, dtype=np.float32)
    W = np.ascontiguousarray(np.asarray(weight, dtype=np.float32))

    # ---- launch 1: wf rows ----
    in1 = _pack_neff1_inputs(nf, lw, lb, W)
    r1 = _run(nc1, in1, cores, trace, "neff1")
    wf = np.concatenate([r1.results[m]["wf_rows"] for m in range(M)], axis=0)

    # ---- launch 2: graph conv ----
    a_pack = _pack_a_fp8(A)
    inv2 = (1.0 / np.square(ncnt.astype(np.float64)))[:, 0].astype(np.float32)
    wfb = wf.astype(BF)
    # wf panels (p, csup, ib, c): wf.reshape(ib, p, csup, cc)
    wfp = np.ascontiguousarray(
        wfb.reshape(32, 128, 8, 512).transpose(1, 2, 0, 3).reshape(128, -1))
    in2 = []
    for m in range(M):
        sl = slice(m * JB, (m + 1) * JB)
        wt = np.ascontiguousarray(wf[sl, :].T) * inv2[:, None]  # [C, JB] f32
        in2.append({
            "ap": a_pack,
            "aot": _pack_cols_kh(np.ascontiguousarray(Ao[:, sl]), F8),
            "sp": _pack_rows_sup(np.ascontiguousarray(S[:, sl]), BF, 8, 4),
            "wfp": wfp,
            "wtp": _pack_rows_sup(wt.astype(np.float32), np.float32, 8, 4),
        })
    r2 = _run(nc2, in2, cores, trace, "neff2")

    out = np.empty((C, N), dtype=np.float32)
    for m in range(M):
        out[:, m * JB:(m + 1) * JB] = r2.results[m]["outc"]
    return out
